# revision 1
# baseline (speedup 1.0000x reference)
"""Trainium2 Bass kernel for an ExponentialRNN (modrelu recurrence).

Computation (per example b):
    xT = x @ T                                   # [B, S, U] pre-projection
    h_{t+1} = modrelu(xT[:, t] + h_t @ B, bias)  # 512 sequential steps
    out[t] = h_{t+1}                             # [S, B, U]

Sharding: data-parallel over batch across 8 cores (8 examples/core).

Per-core device program:
  All inputs arrive as ONE packed [128, W] f32 tensor (single DMA ->
  single DMA-queue semaphore; the f32r matmul LDW slot can carry only
  one sync wait, so multi-queue input fan-in must be avoided).

  Phase 1: pre-projection xT in transposed layout [units, (t, b)]
           via f32r matmuls (T chunks stationary, x^T chunks moving).
  Phase 2: 512 sequential steps. State kept transposed (units on
           partitions) in a rolling SBUF buffer `stb` feeding the matmul
           stationary operand directly:
             psum[8, 512] = sum_k stb[k]^T @ B[k]      (4 f32r matmuls)
             zT = 32x32 stream-transpose(psum)          (4 DVE ops)
             stb[k][:, next] = modrelu(zT + xT_t)       (4 fused custom DVE ops)
           modrelu(z) = sign(z)*relu(|z|+bias) is computed exactly as
             z + clamp(m*z, -c, c),  m = bias>=0 ? BIG : -1,  c = |bias|.
  Output: states stay in transposed layout, DMA'd out in blocks;
          host un-transposes.
"""

import os
import sys

import numpy as np

for _p in ("/opt/trn_rl_repo", "/root/.axon_site/_ro/trn_rl_repo"):
    if os.path.isdir(_p) and _p not in sys.path:
        sys.path.insert(0, _p)

import concourse.bass as bass
import concourse.bacc as bacc
import concourse.mybir as mybir
import concourse.bass_utils as bass_utils
import concourse.dve_ops as dve_ops
from concourse.dve_spec import Spec, Src0, Src1, C0, C1, Zero, maxx, minn, lower
from concourse.dve_uop import DveOpSpec
from concourse.tile import TileContext

BATCH, SEQ, DIN, UNITS = 64, 512, 256, 512
NCORES = 8
BS = BATCH // NCORES          # per-core batch = 8
NK = UNITS // 128             # 4 unit chunks
ND = DIN // 128               # 2 din chunks
F32 = mybir.dt.float32
F32R = mybir.dt.float32r
TB = SEQ * BS                 # flattened (t, b) = 4096


LAST_RESULTS = None


def _register_modrelu():
    """Register the fused modrelu custom DVE op (idempotent).

    out = z + clamp(z*C0, C1, -C1)  with z = Src0 + Src1
    equals sign(z) * relu(|z| + bias) for
      C0 = bias >= 0 ? BIG : -1.0 ,  C1 = |bias|   (per-partition scalars).
    """
    name = "MODRELU_STEP_ANT"
    for op in dve_ops.OPS:
        if op.name == name:
            return op

    z = Src0 + Src1
    spec = Spec(
        body=z + maxx(minn(z * C0, C1), Zero - C1),
        reference=lambda in0, in1, s0, s1, imm2: (in0 + in1)
        + np.maximum(np.minimum((in0 + in1) * s0, s1), -s1),
    )
    shas = {}
    for ver in ("v3", "v4"):
        try:
            uops = lower(spec, ver=ver)
        except Exception:
            continue
        shas[ver] = DveOpSpec(name=name, uops=uops, rd1_en=True).sha(ver)
    op = dve_ops.DveOp(name, spec, subdim=False, uops_sha=shas)
    dve_ops.OPS.append(op)
    row = max(dve_ops._SUB_OPCODE_FOR_NAME.values()) + 1
    assert row < 0x20, "custom DVE opcode rows exhausted"
    dve_ops._SUB_OPCODE_FOR_NAME[name] = row
    dve_ops.CUSTOM_DVE_SPECS[name] = spec
    return op


MODRELU = _register_modrelu()

_NC_CACHE = None


def _build_nc(repeat=1):
    """Build the (SPMD-identical) Bass program for one core.

    repeat>1 wraps phase 2 in a hardware loop (identical passes from h0) —
    used only for device-time slope measurements.
    """
    nc = bacc.Bacc()

    xtr_d = nc.dram_tensor("xtr", [DIN, TB], F32, kind="ExternalInput")
    t_d = nc.dram_tensor("tmat", [DIN, UNITS], F32, kind="ExternalInput")
    # b2: [hi(512) | lo(512)] rows, host-rounded to f32r format
    b2_d = nc.dram_tensor("b2", [2 * UNITS, UNITS], F32, kind="ExternalInput")
    # h02: per chunk k cols [16k..16k+8)=hi, [+8..+16)=lo, host-rounded
    h02_d = nc.dram_tensor("h02", [UNITS, 2 * BS], F32, kind="ExternalInput")
    mv_d = nc.dram_tensor("mv", [UNITS, 1], F32, kind="ExternalInput")
    cv_d = nc.dram_tensor("cv", [UNITS, 1], F32, kind="ExternalInput")
    y_d = nc.dram_tensor("y", [NK, 128, TB], F32, kind="ExternalOutput")

    with TileContext(nc) as tc:
        with (
            tc.tile_pool(name="persist", bufs=1) as pp,
            tc.tile_pool(name="pre_psum", bufs=4, space="PSUM") as pre_ps,
            tc.tile_pool(name="rec_psum", bufs=1, space="PSUM") as rec_ps,
        ):
            # ---- persistent SBUF tensors -------------------------------
            xtr_sb = [pp.tile([128, TB], F32, tag=f"xtr{i}", name=f"xtr{i}")
                      for i in range(ND)]
            t_sb = [pp.tile([128, UNITS], F32, tag=f"t{i}", name=f"t{i}")
                    for i in range(ND)]
            # f32r operand tiles (content host-pre-rounded; the f32r-dest
            # DMA's rounding is then a no-op)
            b2_sb = [pp.tile([128, UNITS], F32R, tag=f"b2_{j}", name=f"b2_{j}")
                     for j in range(2 * NK)]          # 4x hi then 4x lo
            h02_sb = pp.tile([128, 2 * BS * NK], F32R, tag="h02", name="h02")
            mv_sb = pp.tile([128, NK], F32, tag="mv", name="mv_sb")
            cv_sb = pp.tile([128, NK], F32, tag="cv", name="cv_sb")
            # xtt_all: chunk k occupies cols [TB*k, TB*(k+1))
            xtt_all = pp.tile([128, NK * TB], F32, tag="xtt", name="xtt_all")
            # full-precision state archive (also the output buffer):
            # ybuf_all[u', TB*k + 8t+b] = h_{t+1}[b, 128k+u'] in fp32
            ybuf_all = pp.tile([128, NK * TB], F32, tag="ybuf", name="ybuf_all")
            # rolling split-state: chunk k slot p=t%2 at cols 32k+16p..+16
            # ([hi(8)|lo(8)])
            sb2_all = pp.tile([128, 4 * 32], F32R, tag="sb2", name="sb2_all")
            zt_pp = [pp.tile([128, 128], F32, tag=f"zt{i}", name=f"zt{i}")
                     for i in range(2)]
            zs_pp = [pp.tile([128, 32], F32, tag=f"zs{i}", name=f"zs{i}")
                     for i in range(2)]
            # psum[par][h]: half h covers output units 256h..256(h+1)
            ps_pp = [[rec_ps.tile([32, UNITS // 2], F32, tag=f"ps{i}_{h}",
                                  name=f"ps{i}_{h}") for h in range(2)]
                     for i in range(2)]

            xtr_v = [xtr_sb[i][:] for i in range(ND)]
            t_v = [t_sb[i][:] for i in range(ND)]
            bhi_v = [b2_sb[k][:] for k in range(NK)]
            blo_v = [b2_sb[NK + k][:] for k in range(NK)]
            h0_v = [h02_sb[:, 2 * k * BS:2 * (k + 1) * BS] for k in range(NK)]
            mv_v = [mv_sb[:, k:k + 1] for k in range(NK)]
            cv_v = [cv_sb[:, k:k + 1] for k in range(NK)]

            # ---- input DMAs --------------------------------------------
            for i in range(ND):
                nc.sync.dma_start(out=xtr_sb[i][:], in_=xtr_d[128 * i:128 * (i + 1), :])
                nc.sync.dma_start(out=t_sb[i][:], in_=t_d[128 * i:128 * (i + 1), :])
            for j in range(2 * NK):
                nc.sync.dma_start(out=b2_sb[j][:],
                                  in_=b2_d[128 * j:128 * (j + 1), :].bitcast(F32R))
            for k in range(NK):
                nc.sync.dma_start(
                    out=h02_sb[:, 2 * k * BS:2 * (k + 1) * BS],
                    in_=h02_d[128 * k:128 * (k + 1), :].bitcast(F32R))
                nc.sync.dma_start(out=mv_sb[:, k:k + 1], in_=mv_d[128 * k:128 * (k + 1), :])
                nc.sync.dma_start(out=cv_sb[:, k:k + 1], in_=cv_d[128 * k:128 * (k + 1), :])

            # psum rows 16..32 are read by the 32x32 stream transposes but
            # never matmul-written; zero them once.
            for i in range(2):
                for h in range(2):
                    nc.vector.memset(ps_pp[i][h][:], 0.0)
            # DVE touches the packed tile once so later DVE ops (customs
            # reading mv/cv) don't each need a DMA-sem wait.
            warm_sb = pp.tile([128, 2], F32, tag="warm_sb", name="warm_sb")
            nc.vector.tensor_copy(warm_sb[:, 0:1], mv_v[0])
            nc.vector.tensor_copy(warm_sb[:, 1:2], cv_v[0])

            # ---- phase 1: pre-projection xT -> xtt (transposed layout) --
            # xtt[k][u', 8t+b] = sum_d T[d, 128k+u'] * x[b, t, d]
            NJ = TB // 512
            for m in range(NK):
                for j in range(NJ):
                    ps = pre_ps.tile([128, 512], F32, tag="pre", name="pre_ps_t")
                    for i in range(ND):
                        nc.tensor.matmul(
                            ps[:],
                            t_v[i][:, 128 * m:128 * (m + 1)],
                            xtr_v[i][:, 512 * j:512 * (j + 1)],
                            start=(i == 0),
                            stop=(i == ND - 1),
                        )
                    nc.vector.tensor_copy(
                        xtt_all[:, TB * m + 512 * j:TB * m + 512 * (j + 1)], ps[:])

            # ---- phase 2: the 512-step recurrence ----------------------
            # State h is kept as a split pair (hi, lo) of f32r values so the
            # 8-matmul group computes the exact fp32 product:
            #   psum[b]   = hi.B_hi + hi.B_lo      (stationary cols 0..8)
            #   psum[8+b] = lo.B_hi + lo.B_lo      (stationary cols 8..16)
            #   z[b] = psum[b] + psum[8+b] + xt    (DVE, after transpose)
            import contextlib
            loop_cm = (tc.For_i(0, repeat, 1) if repeat > 1
                       else contextlib.nullcontext())
            with loop_cm:
              for t in range(SEQ):
                  zt = zt_pp[t % 2]
                  zs = zs_pp[t % 2]
                  pprev = (t - 1) % 2
                  p = t % 2
                  zt_v = zt[:].rearrange("p (k c) -> p k c", k=NK)
                  zs_v = zs[:].rearrange("p (k c) -> p k c", k=NK)
                  yb_v = ybuf_all[:].rearrange("p (k c) -> p k c", k=NK)
                  sb_v = sb2_all[:].rearrange("p (k c) -> p k c", k=NK)

                  def split_state(ks):
                      """hi = f32r(h), lo = f32r(h - hi) for chunk range ks."""
                      yv = yb_v[:, ks, BS * t:BS * (t + 1)]
                      hv = sb_v[:, ks, 16 * p:16 * p + BS]
                      lv = sb_v[:, ks, 16 * p + BS:16 * p + 2 * BS]
                      nc.vector.tensor_copy(hv, yv)
                      nc.vector.tensor_sub(lv, yv, hv.bitcast(F32))

                  for h in range(2):
                      ps = ps_pp[t % 2][h]
                      cols = slice(256 * h, 256 * (h + 1))
                      # 8 matmuls: all four split products for this half
                      for k in range(NK):
                          lhsT = (h0_v[k] if t == 0
                                  else sb2_all[:, 32 * k + 16 * pprev:
                                               32 * k + 16 * pprev + 16])
                          for i, bp_v in enumerate((bhi_v, blo_v)):
                              nc.tensor.matmul(
                                  ps[0:2 * BS, :],
                                  lhsT,
                                  bp_v[k][:, cols],
                                  start=(k == 0 and i == 0),
                                  stop=(k == NK - 1 and i == 1),
                              )
                      # transpose this half's chunks (2h, 2h+1)
                      ps_v = ps[:].rearrange("p (k c) -> p k c", k=2)
                      for q in range(4):
                          in_ap = ps_v[:, :, 32 * q:32 * (q + 1)]
                          out_ap = zt[32 * q:32 * (q + 1), :].rearrange(
                              "p (k c) -> p k c", k=NK)[:, 2 * h:2 * h + 2, :]
                          nc.vector.transpose(out_ap, in_ap)
                      # pair-add the hi/lo product rows for this half
                      hs = slice(2 * h, 2 * h + 2)
                      nc.vector.tensor_add(
                          zs_v[:, hs, :], zt_v[:, hs, 0:BS],
                          zt_v[:, hs, BS:2 * BS])
                      # h = modrelu(z + xt) (fp32 output archive)
                      for k in (2 * h, 2 * h + 1):
                          nc.vector._custom_dve(
                              MODRELU,
                              out=ybuf_all[:, TB * k + BS * t:TB * k + BS * (t + 1)],
                              in0=zs[:, BS * k:BS * (k + 1)],
                              in1=xtt_all[:, TB * k + BS * t:TB * k + BS * (t + 1)],
                              s0=mv_v[k],
                              s1=cv_v[k],
                          )
                      # split this half's chunks for the next step
                      split_state(hs)
                  # stream finished state blocks out to HBM
                  if (t + 1) % 128 == 0:
                      blk = (t + 1) // 128 - 1
                      lo, hi = 1024 * blk, 1024 * (blk + 1)
                      for k in range(NK):
                          nc.sync.dma_start(
                              out=y_d[k, :, lo:hi],
                              in_=ybuf_all[:, TB * k + lo:TB * k + hi],
                          )

    return nc


def _get_nc():
    global _NC_CACHE
    if _NC_CACHE is None:
        nc = _build_nc()
        nc.finalize()          # run the bacc lowering passes
        _NC_CACHE = nc
    return _NC_CACHE


def _round_f32r(a):
    """Round fp32 values to the f32r format: round-to-nearest-even to 11
    explicit mantissa bits (low 12 bits zero) — HW-verified against the DVE
    f32->f32r rounding copy. Raw fp32 bits fed to an f32r matmul corrupt it."""
    u = np.ascontiguousarray(a, dtype=np.float32).view(np.uint32).copy()
    u += np.uint32(0x7FF) + ((u >> np.uint32(12)) & np.uint32(1))
    u &= np.uint32(0xFFFFF000)
    return u.view(np.float32)


def _pack_inputs(x, T, B, bias, h0):
    """Build the per-core input maps."""
    Bhi = _round_f32r(B)
    Blo = _round_f32r(B - Bhi)
    b2 = np.concatenate([Bhi, Blo], axis=0)               # [1024, 512]
    mv = np.where(bias >= 0, np.float32(1e20), np.float32(-1.0)).astype(np.float32)
    cv = np.abs(bias).astype(np.float32)
    h0b = np.repeat(h0[:, None], BS, axis=1).astype(np.float32)   # [U, BS]
    h0hi = _round_f32r(h0b)
    h0lo = _round_f32r(h0b - h0hi)
    h02 = np.concatenate([h0hi, h0lo], axis=1)            # [U, 16]

    base = {
        "tmat": T,
        "b2": b2,
        "h02": np.ascontiguousarray(h02),
        "mv": mv.reshape(UNITS, 1),
        "cv": cv.reshape(UNITS, 1),
    }
    maps = []
    for c in range(NCORES):
        xs = x[c * BS:(c + 1) * BS]                       # [BS, SEQ, DIN]
        xtr = np.ascontiguousarray(
            xs.transpose(2, 1, 0).reshape(DIN, TB))       # [DIN, (t, b)]
        m = dict(base)
        m["xtr"] = xtr
        maps.append(m)
    return maps


def kernel(x, T, B, bias, h0):
    """Full-input, full-output entry point."""
    global LAST_RESULTS
    x = np.ascontiguousarray(np.asarray(x, dtype=np.float32))
    T = np.ascontiguousarray(np.asarray(T, dtype=np.float32))
    B = np.ascontiguousarray(np.asarray(B, dtype=np.float32))
    bias = np.asarray(bias, dtype=np.float32)
    h0 = np.asarray(h0, dtype=np.float32)

    in_maps = _pack_inputs(x, T, B, bias, h0)

    nc = _get_nc()
    trace = bool(int(os.environ.get("KERNEL_TRACE", "0")))
    res = bass_utils.run_bass_kernel_spmd(
        nc, in_maps, list(range(NCORES)), trace=trace)
    LAST_RESULTS = res

    out = np.empty((SEQ, BATCH, UNITS), dtype=np.float32)
    for c in range(NCORES):
        y = res.results[c]["y"].reshape(NK, 128, SEQ, BS)
        # out[t, b, 128k+p] = y[k, p, t, b]
        out[:, c * BS:(c + 1) * BS, :] = (
            y.transpose(2, 3, 0, 1).reshape(SEQ, BS, UNITS))
    return out


if __name__ == "__main__":
    rng = np.random.default_rng(0)
    x = rng.standard_normal((BATCH, SEQ, DIN), dtype=np.float32)
    T = rng.standard_normal((DIN, UNITS), dtype=np.float32) / DIN
    B = rng.standard_normal((UNITS, UNITS), dtype=np.float32) / 22.0
    bias = rng.uniform(-0.01, 0.01, UNITS).astype(np.float32)
    h0 = np.zeros(UNITS, dtype=np.float32)
    out = kernel(x=x, T=T, B=B, bias=bias, h0=h0)
    print("out", out.shape, out.dtype, float(np.abs(out).mean()))



# revision 2
# speedup vs baseline: 3.6019x; 3.6019x over previous
"""Trainium2 Bass kernel for an ExponentialRNN (modrelu recurrence).

Computation (per example b):
    xT = x @ T                                   # [B, S, U] pre-projection
    h_{t+1} = modrelu(xT[:, t] + h_t @ B, bias)  # 512 sequential steps
    out[t] = h_{t+1}                             # [S, B, U]

Sharding: data-parallel over batch across 8 cores (8 examples/core).

Fast path (used for the reference inputs): B = expm(A) with A built from a
2x2-block skew-symmetric structure, so B is EXACTLY block-diagonal with
2x2 rotation blocks acting on unit pairs (2p, 2p+1).  h @ B is then an
elementwise pair rotation, not a matmul:

    z[2p]   = h[2p]*B[2p,2p]   + h[2p+1]*B[2p+1,2p]
    z[2p+1] = h[2p]*B[2p,2p+1] + h[2p+1]*B[2p+1,2p+1]

Device program (fast path), per core:
  Units are regrouped on host into 4 groups of 128 (partition layout):
    g=0: even units of pairs 0..127    g=1: odd units of pairs 0..127
    g=2: even units of pairs 128..255  g=3: odd units of pairs 128..255
  Phase 1 (PE + Act): xtt[g][q, 8t+b] = (x @ T)[b, t, unit(g,q)] via f32
    matmuls with column-permuted T; psum->sbuf copies on the Act engine so
    the DVE stream stays dedicated to the recurrence.
  Recurrence (DVE only, 8 fused ops/step, all SBUF):
    for j in {0,1}:  (ge, go) = (2j, 2j+1)
      z[ge] = w[ge]*A00_j + w[go]*A10_j          (ROT custom op)
      z[go] = w[ge]*A01_j + w[go]*A11_j          (ROT custom op)
      w'[ge] = modrelu(z[ge] + xtt[ge][t])       (MODRELU custom op)
      w'[go] = modrelu(z[go] + xtt[go][t])       (MODRELU custom op)
    modrelu(z) = sign(z)*relu(|z|+bias) computed exactly as
      z + clamp(m*z, -c, c),  m = bias>=0 ? BIG : -1,  c = |bias|.
    State lives directly in the output archive ybuf[g][:, 8t:8t+8] (fp32),
    so no extra copies; same-engine program order needs no semaphores.
  Output: ybuf blocks DMA'd out every 64 steps; host un-permutes.

Fallback (B not block-diagonal): dense f32r split-precision matmul
recurrence (the previous kernel), kept verbatim below.
"""

import os
import sys

import numpy as np

for _p in ("/opt/trn_rl_repo", "/root/.axon_site/_ro/trn_rl_repo"):
    if os.path.isdir(_p) and _p not in sys.path:
        sys.path.insert(0, _p)

import concourse.bass as bass
import concourse.bacc as bacc
import concourse.mybir as mybir
import concourse.bass_utils as bass_utils
import concourse.dve_ops as dve_ops
from concourse.dve_spec import Spec, Src0, Src1, C0, C1, Zero, maxx, minn, lower
from concourse.dve_uop import DveOpSpec
from concourse.tile import TileContext

BATCH, SEQ, DIN, UNITS = 64, 512, 256, 512
NCORES = 8
BS = BATCH // NCORES          # per-core batch = 8
NK = UNITS // 128             # 4 unit chunks
ND = DIN // 128               # 2 din chunks
F32 = mybir.dt.float32
F32R = mybir.dt.float32r
TB = SEQ * BS                 # flattened (t, b) = 4096
NPAIR = UNITS // 2            # 256 rotation pairs
NG = 4                        # unit groups of 128 (even/odd x pair-chunk)


LAST_RESULTS = None


def _register_dve_op(name, body, ref):
    """Register a custom DVE op (idempotent)."""
    for op in dve_ops.OPS:
        if op.name == name:
            return op
    spec = Spec(body=body, reference=ref)
    shas = {}
    for ver in ("v3", "v4"):
        try:
            uops = lower(spec, ver=ver)
        except Exception:
            continue
        shas[ver] = DveOpSpec(name=name, uops=uops, rd1_en=True).sha(ver)
    op = dve_ops.DveOp(name, spec, subdim=False, uops_sha=shas)
    dve_ops.OPS.append(op)
    row = max(dve_ops._SUB_OPCODE_FOR_NAME.values()) + 1
    assert row < 0x20, "custom DVE opcode rows exhausted"
    dve_ops._SUB_OPCODE_FOR_NAME[name] = row
    dve_ops.CUSTOM_DVE_SPECS[name] = spec
    return op


def _register_modrelu():
    """out = z + clamp(z*C0, C1, -C1)  with z = Src0 + Src1
    equals sign(z) * relu(|z| + bias) for
      C0 = bias >= 0 ? BIG : -1.0 ,  C1 = |bias|   (per-partition scalars)."""
    z = Src0 + Src1
    return _register_dve_op(
        "MODRELU_STEP_ANT",
        z + maxx(minn(z * C0, C1), Zero - C1),
        lambda in0, in1, s0, s1, imm2: (in0 + in1)
        + np.maximum(np.minimum((in0 + in1) * s0, s1), -s1),
    )


def _register_rot():
    """out = Src0*C0 + Src1*C1 — one half of a 2x2 pair rotation."""
    return _register_dve_op(
        "PAIR_ROT_ANT",
        Src0 * C0 + Src1 * C1,
        lambda in0, in1, s0, s1, imm2: in0 * s0 + in1 * s1,
    )


MODRELU = _register_modrelu()
ROT = _register_rot()

_NC_CACHE = {}


# ---------------------------------------------------------------------------
# Fast path: block-diagonal rotation recurrence (pure DVE)
# ---------------------------------------------------------------------------

def _build_nc_rot():
    nc = bacc.Bacc()

    xtr_d = nc.dram_tensor("xtr", [DIN, TB], F32, kind="ExternalInput")
    t_d = nc.dram_tensor("tmat", [DIN, UNITS], F32, kind="ExternalInput")
    scal_d = nc.dram_tensor("scal", [128, 16], F32, kind="ExternalInput")
    h0_d = nc.dram_tensor("h0t", [128, NG * BS], F32, kind="ExternalInput")
    y_d = nc.dram_tensor("y", [NG, 128, TB], F32, kind="ExternalOutput")

    NJ = TB // 512            # 8 tb-blocks of 512 cols (= 64 steps)

    with TileContext(nc) as tc:
        with (
            tc.tile_pool(name="persist", bufs=1) as pp,
            tc.tile_pool(name="pre_psum", bufs=4, space="PSUM") as pre_ps,
        ):
            xtr_sb = [pp.tile([128, TB], F32, tag=f"xtr{i}", name=f"xtr{i}")
                      for i in range(ND)]
            t_sb = [pp.tile([128, UNITS], F32, tag=f"t{i}", name=f"t{i}")
                    for i in range(ND)]
            scal_sb = pp.tile([128, 16], F32, tag="scal", name="scal_sb")
            h0_sb = pp.tile([128, NG * BS], F32, tag="h0t", name="h0_sb")
            xtt = [pp.tile([128, TB], F32, tag=f"xtt{g}", name=f"xtt{g}")
                   for g in range(NG)]
            ybuf = [pp.tile([128, TB], F32, tag=f"ybuf{g}", name=f"ybuf{g}")
                    for g in range(NG)]
            zt = [pp.tile([128, BS], F32, tag=f"z{g}", name=f"z{g}")
                  for g in range(NG)]

            # scal columns: per pair-chunk j: [4j+0..4j+3] = A00, A10, A01, A11
            # per group g: [8+g] = m (modrelu selector), [12+g] = |bias|
            A = [[scal_sb[:, 4 * j + k:4 * j + k + 1] for k in range(4)]
                 for j in range(2)]
            mv = [scal_sb[:, 8 + g:9 + g] for g in range(NG)]
            cv = [scal_sb[:, 12 + g:13 + g] for g in range(NG)]

            # ---- input DMAs -------------------------------------------
            for i in range(ND):
                nc.sync.dma_start(out=xtr_sb[i][:],
                                  in_=xtr_d[128 * i:128 * (i + 1), :])
                nc.sync.dma_start(out=t_sb[i][:],
                                  in_=t_d[128 * i:128 * (i + 1), :])
            nc.sync.dma_start(out=scal_sb[:], in_=scal_d[:, :])
            nc.sync.dma_start(out=h0_sb[:], in_=h0_d[:, :])

            # ---- phase 1: pre-projection into group layout ------------
            # xtt[g][q, col] = sum_d Tp[d, 128g+q] * xtr[d, col]
            for g in range(NG):
                for bj in range(NJ):
                    ps = pre_ps.tile([128, 512], F32, tag="pre",
                                     name="pre_ps_t")
                    for i in range(ND):
                        nc.tensor.matmul(
                            ps[:],
                            t_sb[i][:, 128 * g:128 * (g + 1)],
                            xtr_sb[i][:, 512 * bj:512 * (bj + 1)],
                            start=(i == 0),
                            stop=(i == ND - 1),
                        )
                    nc.scalar.copy(
                        xtt[g][:, 512 * bj:512 * (bj + 1)], ps[:])

            # ---- recurrence: 8 DVE ops per step, all SBUF -------------
            for t in range(SEQ):
                cols = slice(BS * t, BS * (t + 1))
                pcols = slice(BS * (t - 1), BS * t)
                for j in range(2):
                    ge, go = 2 * j, 2 * j + 1
                    if t == 0:
                        we = h0_sb[:, BS * ge:BS * (ge + 1)]
                        wo = h0_sb[:, BS * go:BS * (go + 1)]
                    else:
                        we = ybuf[ge][:, pcols]
                        wo = ybuf[go][:, pcols]
                    nc.vector._custom_dve(
                        ROT, out=zt[ge][:], in0=we, in1=wo,
                        s0=A[j][0], s1=A[j][1])
                    nc.vector._custom_dve(
                        ROT, out=zt[go][:], in0=we, in1=wo,
                        s0=A[j][2], s1=A[j][3])
                    nc.vector._custom_dve(
                        MODRELU, out=ybuf[ge][:, cols],
                        in0=zt[ge][:], in1=xtt[ge][:, cols],
                        s0=mv[ge], s1=cv[ge])
                    nc.vector._custom_dve(
                        MODRELU, out=ybuf[go][:, cols],
                        in0=zt[go][:], in1=xtt[go][:, cols],
                        s0=mv[go], s1=cv[go])
                # stream finished 512-col blocks out to HBM
                if (t + 1) % 64 == 0:
                    blk = (t + 1) // 64 - 1
                    lo, hi = 512 * blk, 512 * (blk + 1)
                    for g in range(NG):
                        nc.sync.dma_start(
                            out=y_d[g, :, lo:hi],
                            in_=ybuf[g][:, lo:hi],
                        )

    return nc


def _unit_perm():
    """perm[128*g + q] = original unit index held by (group g, partition q)."""
    q = np.arange(128)
    return np.concatenate([2 * q, 2 * q + 1, 256 + 2 * q, 257 + 2 * q])


def _pack_inputs_rot(x, T, B, bias, h0):
    perm = _unit_perm()
    Tp = np.ascontiguousarray(T[:, perm])                 # [DIN, UNITS]

    p = np.arange(NPAIR)
    A00 = B[2 * p, 2 * p]
    A10 = B[2 * p + 1, 2 * p]
    A01 = B[2 * p, 2 * p + 1]
    A11 = B[2 * p + 1, 2 * p + 1]

    scal = np.zeros((128, 16), dtype=np.float32)
    for j in range(2):
        sl = slice(128 * j, 128 * (j + 1))
        scal[:, 4 * j + 0] = A00[sl]
        scal[:, 4 * j + 1] = A10[sl]
        scal[:, 4 * j + 2] = A01[sl]
        scal[:, 4 * j + 3] = A11[sl]
    bp = bias[perm].reshape(NG, 128)
    for g in range(NG):
        scal[:, 8 + g] = np.where(bp[g] >= 0, np.float32(1e20),
                                  np.float32(-1.0))
        scal[:, 12 + g] = np.abs(bp[g])

    h0t = np.repeat(h0[perm].reshape(NG, 128).T.reshape(128, NG, 1),
                    BS, axis=2).reshape(128, NG * BS).astype(np.float32)

    base = {"tmat": Tp, "scal": scal, "h0t": np.ascontiguousarray(h0t)}
    maps = []
    for c in range(NCORES):
        xs = x[c * BS:(c + 1) * BS]                       # [BS, SEQ, DIN]
        xtr = np.ascontiguousarray(
            xs.transpose(2, 1, 0).reshape(DIN, TB))       # [DIN, (t, b)]
        m = dict(base)
        m["xtr"] = xtr
        maps.append(m)
    return maps


def _unpack_rot(res):
    perm = _unit_perm()
    out = np.empty((SEQ, BATCH, UNITS), dtype=np.float32)
    for c in range(NCORES):
        y = res.results[c]["y"].reshape(NG, 128, SEQ, BS)
        # out[t, b, perm[128g+q]] = y[g, q, t, b]
        tmp = y.transpose(2, 3, 0, 1).reshape(SEQ, BS, UNITS)
        out[:, c * BS:(c + 1) * BS, perm] = tmp
    return out


def _is_block_diag(B):
    Babs = np.abs(B)
    mask = np.zeros((UNITS, UNITS), dtype=bool)
    p = np.arange(NPAIR)
    for (i, j) in ((0, 0), (0, 1), (1, 0), (1, 1)):
        mask[2 * p + i, 2 * p + j] = True
    off = Babs[~mask]
    return off.max() <= 1e-5 * max(Babs.max(), 1e-30)


# ---------------------------------------------------------------------------
# Fallback: dense f32r split-precision matmul recurrence (previous kernel)
# ---------------------------------------------------------------------------

def _build_nc_dense(repeat=1):
    nc = bacc.Bacc()

    xtr_d = nc.dram_tensor("xtr", [DIN, TB], F32, kind="ExternalInput")
    t_d = nc.dram_tensor("tmat", [DIN, UNITS], F32, kind="ExternalInput")
    b2_d = nc.dram_tensor("b2", [2 * UNITS, UNITS], F32, kind="ExternalInput")
    h02_d = nc.dram_tensor("h02", [UNITS, 2 * BS], F32, kind="ExternalInput")
    mv_d = nc.dram_tensor("mv", [UNITS, 1], F32, kind="ExternalInput")
    cv_d = nc.dram_tensor("cv", [UNITS, 1], F32, kind="ExternalInput")
    y_d = nc.dram_tensor("y", [NK, 128, TB], F32, kind="ExternalOutput")

    with TileContext(nc) as tc:
        with (
            tc.tile_pool(name="persist", bufs=1) as pp,
            tc.tile_pool(name="pre_psum", bufs=4, space="PSUM") as pre_ps,
            tc.tile_pool(name="rec_psum", bufs=1, space="PSUM") as rec_ps,
        ):
            xtr_sb = [pp.tile([128, TB], F32, tag=f"xtr{i}", name=f"xtr{i}")
                      for i in range(ND)]
            t_sb = [pp.tile([128, UNITS], F32, tag=f"t{i}", name=f"t{i}")
                    for i in range(ND)]
            b2_sb = [pp.tile([128, UNITS], F32R, tag=f"b2_{j}", name=f"b2_{j}")
                     for j in range(2 * NK)]
            h02_sb = pp.tile([128, 2 * BS * NK], F32R, tag="h02", name="h02")
            mv_sb = pp.tile([128, NK], F32, tag="mv", name="mv_sb")
            cv_sb = pp.tile([128, NK], F32, tag="cv", name="cv_sb")
            xtt_all = pp.tile([128, NK * TB], F32, tag="xtt", name="xtt_all")
            ybuf_all = pp.tile([128, NK * TB], F32, tag="ybuf", name="ybuf_all")
            sb2_all = pp.tile([128, 4 * 32], F32R, tag="sb2", name="sb2_all")
            zt_pp = [pp.tile([128, 128], F32, tag=f"zt{i}", name=f"zt{i}")
                     for i in range(2)]
            zs_pp = [pp.tile([128, 32], F32, tag=f"zs{i}", name=f"zs{i}")
                     for i in range(2)]
            ps_pp = [[rec_ps.tile([32, UNITS // 2], F32, tag=f"ps{i}_{h}",
                                  name=f"ps{i}_{h}") for h in range(2)]
                     for i in range(2)]

            xtr_v = [xtr_sb[i][:] for i in range(ND)]
            t_v = [t_sb[i][:] for i in range(ND)]
            bhi_v = [b2_sb[k][:] for k in range(NK)]
            blo_v = [b2_sb[NK + k][:] for k in range(NK)]
            h0_v = [h02_sb[:, 2 * k * BS:2 * (k + 1) * BS] for k in range(NK)]
            mv_v = [mv_sb[:, k:k + 1] for k in range(NK)]
            cv_v = [cv_sb[:, k:k + 1] for k in range(NK)]

            for i in range(ND):
                nc.sync.dma_start(out=xtr_sb[i][:], in_=xtr_d[128 * i:128 * (i + 1), :])
                nc.sync.dma_start(out=t_sb[i][:], in_=t_d[128 * i:128 * (i + 1), :])
            for j in range(2 * NK):
                nc.sync.dma_start(out=b2_sb[j][:],
                                  in_=b2_d[128 * j:128 * (j + 1), :].bitcast(F32R))
            for k in range(NK):
                nc.sync.dma_start(
                    out=h02_sb[:, 2 * k * BS:2 * (k + 1) * BS],
                    in_=h02_d[128 * k:128 * (k + 1), :].bitcast(F32R))
                nc.sync.dma_start(out=mv_sb[:, k:k + 1], in_=mv_d[128 * k:128 * (k + 1), :])
                nc.sync.dma_start(out=cv_sb[:, k:k + 1], in_=cv_d[128 * k:128 * (k + 1), :])

            for i in range(2):
                for h in range(2):
                    nc.vector.memset(ps_pp[i][h][:], 0.0)
            warm_sb = pp.tile([128, 2], F32, tag="warm_sb", name="warm_sb")
            nc.vector.tensor_copy(warm_sb[:, 0:1], mv_v[0])
            nc.vector.tensor_copy(warm_sb[:, 1:2], cv_v[0])

            NJ = TB // 512
            for m in range(NK):
                for j in range(NJ):
                    ps = pre_ps.tile([128, 512], F32, tag="pre", name="pre_ps_t")
                    for i in range(ND):
                        nc.tensor.matmul(
                            ps[:],
                            t_v[i][:, 128 * m:128 * (m + 1)],
                            xtr_v[i][:, 512 * j:512 * (j + 1)],
                            start=(i == 0),
                            stop=(i == ND - 1),
                        )
                    nc.vector.tensor_copy(
                        xtt_all[:, TB * m + 512 * j:TB * m + 512 * (j + 1)], ps[:])

            import contextlib
            loop_cm = (tc.For_i(0, repeat, 1) if repeat > 1
                       else contextlib.nullcontext())
            with loop_cm:
              for t in range(SEQ):
                  zt = zt_pp[t % 2]
                  zs = zs_pp[t % 2]
                  pprev = (t - 1) % 2
                  p = t % 2
                  zt_v = zt[:].rearrange("p (k c) -> p k c", k=NK)
                  zs_v = zs[:].rearrange("p (k c) -> p k c", k=NK)
                  yb_v = ybuf_all[:].rearrange("p (k c) -> p k c", k=NK)
                  sb_v = sb2_all[:].rearrange("p (k c) -> p k c", k=NK)

                  def split_state(ks):
                      yv = yb_v[:, ks, BS * t:BS * (t + 1)]
                      hv = sb_v[:, ks, 16 * p:16 * p + BS]
                      lv = sb_v[:, ks, 16 * p + BS:16 * p + 2 * BS]
                      nc.vector.tensor_copy(hv, yv)
                      nc.vector.tensor_sub(lv, yv, hv.bitcast(F32))

                  for h in range(2):
                      ps = ps_pp[t % 2][h]
                      cols = slice(256 * h, 256 * (h + 1))
                      for k in range(NK):
                          lhsT = (h0_v[k] if t == 0
                                  else sb2_all[:, 32 * k + 16 * pprev:
                                               32 * k + 16 * pprev + 16])
                          for i, bp_v in enumerate((bhi_v, blo_v)):
                              nc.tensor.matmul(
                                  ps[0:2 * BS, :],
                                  lhsT,
                                  bp_v[k][:, cols],
                                  start=(k == 0 and i == 0),
                                  stop=(k == NK - 1 and i == 1),
                              )
                      ps_v = ps[:].rearrange("p (k c) -> p k c", k=2)
                      for q in range(4):
                          in_ap = ps_v[:, :, 32 * q:32 * (q + 1)]
                          out_ap = zt[32 * q:32 * (q + 1), :].rearrange(
                              "p (k c) -> p k c", k=NK)[:, 2 * h:2 * h + 2, :]
                          nc.vector.transpose(out_ap, in_ap)
                      hs = slice(2 * h, 2 * h + 2)
                      nc.vector.tensor_add(
                          zs_v[:, hs, :], zt_v[:, hs, 0:BS],
                          zt_v[:, hs, BS:2 * BS])
                      for k in (2 * h, 2 * h + 1):
                          nc.vector._custom_dve(
                              MODRELU,
                              out=ybuf_all[:, TB * k + BS * t:TB * k + BS * (t + 1)],
                              in0=zs[:, BS * k:BS * (k + 1)],
                              in1=xtt_all[:, TB * k + BS * t:TB * k + BS * (t + 1)],
                              s0=mv_v[k],
                              s1=cv_v[k],
                          )
                      split_state(hs)
                  if (t + 1) % 128 == 0:
                      blk = (t + 1) // 128 - 1
                      lo, hi = 1024 * blk, 1024 * (blk + 1)
                      for k in range(NK):
                          nc.sync.dma_start(
                              out=y_d[k, :, lo:hi],
                              in_=ybuf_all[:, TB * k + lo:TB * k + hi],
                          )

    return nc


def _round_f32r(a):
    u = np.ascontiguousarray(a, dtype=np.float32).view(np.uint32).copy()
    u += np.uint32(0x7FF) + ((u >> np.uint32(12)) & np.uint32(1))
    u &= np.uint32(0xFFFFF000)
    return u.view(np.float32)


def _pack_inputs_dense(x, T, B, bias, h0):
    Bhi = _round_f32r(B)
    Blo = _round_f32r(B - Bhi)
    b2 = np.concatenate([Bhi, Blo], axis=0)
    mv = np.where(bias >= 0, np.float32(1e20), np.float32(-1.0)).astype(np.float32)
    cv = np.abs(bias).astype(np.float32)
    h0b = np.repeat(h0[:, None], BS, axis=1).astype(np.float32)
    h0hi = _round_f32r(h0b)
    h0lo = _round_f32r(h0b - h0hi)
    h02 = np.concatenate([h0hi, h0lo], axis=1)

    base = {
        "tmat": T,
        "b2": b2,
        "h02": np.ascontiguousarray(h02),
        "mv": mv.reshape(UNITS, 1),
        "cv": cv.reshape(UNITS, 1),
    }
    maps = []
    for c in range(NCORES):
        xs = x[c * BS:(c + 1) * BS]
        xtr = np.ascontiguousarray(
            xs.transpose(2, 1, 0).reshape(DIN, TB))
        m = dict(base)
        m["xtr"] = xtr
        maps.append(m)
    return maps


def _unpack_dense(res):
    out = np.empty((SEQ, BATCH, UNITS), dtype=np.float32)
    for c in range(NCORES):
        y = res.results[c]["y"].reshape(NK, 128, SEQ, BS)
        out[:, c * BS:(c + 1) * BS, :] = (
            y.transpose(2, 3, 0, 1).reshape(SEQ, BS, UNITS))
    return out


# ---------------------------------------------------------------------------

def _get_nc(kind):
    if kind not in _NC_CACHE:
        nc = _build_nc_rot() if kind == "rot" else _build_nc_dense()
        nc.finalize()
        _NC_CACHE[kind] = nc
    return _NC_CACHE[kind]


def kernel(x, T, B, bias, h0):
    """Full-input, full-output entry point."""
    global LAST_RESULTS
    x = np.ascontiguousarray(np.asarray(x, dtype=np.float32))
    T = np.ascontiguousarray(np.asarray(T, dtype=np.float32))
    B = np.ascontiguousarray(np.asarray(B, dtype=np.float32))
    bias = np.asarray(bias, dtype=np.float32)
    h0 = np.asarray(h0, dtype=np.float32)

    kind = "rot" if _is_block_diag(B) else "dense"
    if kind == "rot":
        in_maps = _pack_inputs_rot(x, T, B, bias, h0)
    else:
        in_maps = _pack_inputs_dense(x, T, B, bias, h0)

    nc = _get_nc(kind)
    trace = bool(int(os.environ.get("KERNEL_TRACE", "0")))
    res = bass_utils.run_bass_kernel_spmd(
        nc, in_maps, list(range(NCORES)), trace=trace)
    LAST_RESULTS = res

    return _unpack_rot(res) if kind == "rot" else _unpack_dense(res)


if __name__ == "__main__":
    rng = np.random.default_rng(0)
    x = rng.standard_normal((BATCH, SEQ, DIN), dtype=np.float32)
    T = rng.standard_normal((DIN, UNITS), dtype=np.float32) / DIN
    # block-diagonal rotation B
    th = rng.uniform(0, np.pi / 2, NPAIR).astype(np.float32)
    B = np.zeros((UNITS, UNITS), dtype=np.float32)
    p = np.arange(NPAIR)
    B[2 * p, 2 * p] = np.cos(th)
    B[2 * p, 2 * p + 1] = np.sin(th)
    B[2 * p + 1, 2 * p] = -np.sin(th)
    B[2 * p + 1, 2 * p + 1] = np.cos(th)
    bias = rng.uniform(-0.01, 0.01, UNITS).astype(np.float32)
    h0 = np.zeros(UNITS, dtype=np.float32)
    out = kernel(x=x, T=T, B=B, bias=bias, h0=h0)
    print("out", out.shape, out.dtype, float(np.abs(out).mean()))


# revision 3
# speedup vs baseline: 6.0787x; 1.6876x over previous
"""Trainium2 Bass kernel for an ExponentialRNN (modrelu recurrence).

Computation (per example b):
    xT = x @ T                                   # [B, S, U] pre-projection
    h_{t+1} = modrelu(xT[:, t] + h_t @ B, bias)  # 512 sequential steps
    out[t] = h_{t+1}                             # [S, B, U]

Sharding: data-parallel over batch across 8 cores (8 examples/core).

Fast path (used for the reference inputs): B = expm(A) with A built from a
2x2-block skew-symmetric structure, so B is EXACTLY block-diagonal with
2x2 rotation blocks acting on unit pairs (2p, 2p+1).  h @ B is then an
elementwise pair rotation, not a matmul:

    z[2p]   = h[2p]*B[2p,2p]   + h[2p+1]*B[2p+1,2p]
    z[2p+1] = h[2p]*B[2p,2p+1] + h[2p+1]*B[2p+1,2p+1]

Device program (fast path), per core:
  Units are regrouped on host into 4 groups of 128 (partition layout):
    g=0: even units of pairs 0..127    g=1: odd units of pairs 0..127
    g=2: even units of pairs 128..255  g=3: odd units of pairs 128..255
  Phase 1 (PE + Act): xtt[g][q, 8t+b] = (x @ T)[b, t, unit(g,q)] via f32
    matmuls with column-permuted T; psum->sbuf copies on the Act engine so
    the DVE stream stays dedicated to the recurrence.
  Recurrence (DVE only, 8 fused ops/step, all SBUF):
    for j in {0,1}:  (ge, go) = (2j, 2j+1)
      z[ge] = w[ge]*A00_j + w[go]*A10_j          (ROT custom op)
      z[go] = w[ge]*A01_j + w[go]*A11_j          (ROT custom op)
      w'[ge] = modrelu(z[ge] + xtt[ge][t])       (MODRELU custom op)
      w'[go] = modrelu(z[go] + xtt[go][t])       (MODRELU custom op)
    modrelu(z) = sign(z)*relu(|z|+bias) computed exactly as
      z + clamp(m*z, -c, c),  m = bias>=0 ? BIG : -1,  c = |bias|.
    State lives directly in the output archive ybuf[g][:, 8t:8t+8] (fp32),
    so no extra copies; same-engine program order needs no semaphores.
  Output: ybuf blocks DMA'd out every 64 steps; host un-permutes.

Fallback (B not block-diagonal): dense f32r split-precision matmul
recurrence (the previous kernel), kept verbatim below.
"""

import os
import sys

import numpy as np

for _p in ("/opt/trn_rl_repo", "/root/.axon_site/_ro/trn_rl_repo"):
    if os.path.isdir(_p) and _p not in sys.path:
        sys.path.insert(0, _p)

import concourse.bass as bass
import concourse.bacc as bacc
import concourse.mybir as mybir
import concourse.bass_utils as bass_utils
import concourse.dve_ops as dve_ops
from concourse.dve_spec import Spec, Src0, Src1, C0, C1, Zero, maxx, minn, lower
from concourse.dve_uop import DveOpSpec
from concourse.tile import TileContext

BATCH, SEQ, DIN, UNITS = 64, 512, 256, 512
NCORES = 8
BS = BATCH // NCORES          # per-core batch = 8
NK = UNITS // 128             # 4 unit chunks
ND = DIN // 128               # 2 din chunks
F32 = mybir.dt.float32
F32R = mybir.dt.float32r
TB = SEQ * BS                 # flattened (t, b) = 4096
NPAIR = UNITS // 2            # 256 rotation pairs
NG = 4                        # unit groups of 128 (even/odd x pair-chunk)


LAST_RESULTS = None


def _register_dve_op(name, body, ref):
    """Register a custom DVE op (idempotent)."""
    for op in dve_ops.OPS:
        if op.name == name:
            return op
    spec = Spec(body=body, reference=ref)
    shas = {}
    for ver in ("v3", "v4"):
        try:
            uops = lower(spec, ver=ver)
        except Exception:
            continue
        shas[ver] = DveOpSpec(name=name, uops=uops, rd1_en=True).sha(ver)
    op = dve_ops.DveOp(name, spec, subdim=False, uops_sha=shas)
    dve_ops.OPS.append(op)
    row = max(dve_ops._SUB_OPCODE_FOR_NAME.values()) + 1
    assert row < 0x20, "custom DVE opcode rows exhausted"
    dve_ops._SUB_OPCODE_FOR_NAME[name] = row
    dve_ops.CUSTOM_DVE_SPECS[name] = spec
    return op


def _register_modrelu():
    """out = z + clamp(z*C0, C1, -C1)  with z = Src0 + Src1
    equals sign(z) * relu(|z| + bias) for
      C0 = bias >= 0 ? BIG : -1.0 ,  C1 = |bias|   (per-partition scalars)."""
    z = Src0 + Src1
    return _register_dve_op(
        "MODRELU_STEP_ANT",
        z + maxx(minn(z * C0, C1), Zero - C1),
        lambda in0, in1, s0, s1, imm2: (in0 + in1)
        + np.maximum(np.minimum((in0 + in1) * s0, s1), -s1),
    )


def _register_rot():
    """out = Src0*C0 + Src1*C1 — one half of a 2x2 pair rotation."""
    return _register_dve_op(
        "PAIR_ROT_ANT",
        Src0 * C0 + Src1 * C1,
        lambda in0, in1, s0, s1, imm2: in0 * s0 + in1 * s1,
    )


MODRELU = _register_modrelu()
ROT = _register_rot()

_NC_CACHE = {}


# ---------------------------------------------------------------------------
# Fast path: block-diagonal rotation recurrence (pure DVE)
#
# Sharding: batch 4-way x pairs 2-way across the 8 cores.  Core c handles
# batch shard cb = c // 2 (16 examples) and pair shard cp = c % 2 (128
# pairs = 1 full partition chunk), so each recurrence step is exactly 4
# DVE ops of [128, 16]: rot-e, rot-o, modrelu-e, modrelu-o.
# ---------------------------------------------------------------------------

NBS = 2                       # pair shards
BSR = BATCH // (NCORES // NBS)  # examples per core = 16
TBR = SEQ * BSR               # per-core flattened (t, b) = 8192
NJR = TBR // 512              # 16 col-blocks of 512 (= 32 steps each)


def _build_nc_rot():
    nc = bacc.Bacc()

    xtr_d = nc.dram_tensor("xtr", [DIN, TBR], F32, kind="ExternalInput")
    t_d = nc.dram_tensor("tmat", [DIN, 256], F32, kind="ExternalInput")
    scal_d = nc.dram_tensor("scal", [128, 8], F32, kind="ExternalInput")
    h0_d = nc.dram_tensor("h0t", [128, 2 * BSR], F32, kind="ExternalInput")
    y_d = nc.dram_tensor("y", [2, 128, TBR], F32, kind="ExternalOutput")

    with TileContext(nc) as tc:
        with (
            tc.tile_pool(name="persist", bufs=1) as pp,
            tc.tile_pool(name="xstream", bufs=4) as xp,
            tc.tile_pool(name="pre_psum", bufs=4, space="PSUM") as pre_ps,
        ):
            t_sb = [pp.tile([128, 256], F32, tag=f"t{i}", name=f"t{i}")
                    for i in range(ND)]
            scal_sb = pp.tile([128, 8], F32, tag="scal", name="scal_sb")
            h0_sb = pp.tile([128, 2 * BSR], F32, tag="h0t", name="h0_sb")
            xtt = [pp.tile([128, TBR], F32, tag=f"xtt{g}", name=f"xtt{g}")
                   for g in range(2)]
            ybuf = [pp.tile([128, TBR], F32, tag=f"ybuf{g}", name=f"ybuf{g}")
                    for g in range(2)]
            zt = [pp.tile([128, BSR], F32, tag=f"z{g}", name=f"z{g}")
                  for g in range(2)]

            # scal columns: 0..3 = A00, A10, A01, A11;
            # 4,5 = m_e, m_o; 6,7 = cb_e, cb_o
            A = [scal_sb[:, k:k + 1] for k in range(4)]
            mv = [scal_sb[:, 4 + g:5 + g] for g in range(2)]
            cv = [scal_sb[:, 6 + g:7 + g] for g in range(2)]

            # ---- small input DMAs -------------------------------------
            for i in range(ND):
                nc.sync.dma_start(out=t_sb[i][:],
                                  in_=t_d[128 * i:128 * (i + 1), :])
            nc.sync.dma_start(out=scal_sb[:], in_=scal_d[:, :])
            nc.sync.dma_start(out=h0_sb[:], in_=h0_d[:, :])

            # ---- phase 1: stream x in, project into group layout ------
            # xtt[g][q, col] = sum_d Tp[d, 128g+q] * x[d, col]
            for bj in range(NJR):
                xs = [xp.tile([128, 512], F32, tag=f"xs{i}",
                              name=f"xs_t{i}") for i in range(ND)]
                for i in range(ND):
                    nc.sync.dma_start(
                        out=xs[i][:],
                        in_=xtr_d[128 * i:128 * (i + 1),
                                  512 * bj:512 * (bj + 1)])
                for g in range(2):
                    ps = pre_ps.tile([128, 512], F32, tag="pre",
                                     name="pre_ps_t")
                    for i in range(ND):
                        nc.tensor.matmul(
                            ps[:],
                            t_sb[i][:, 128 * g:128 * (g + 1)],
                            xs[i][:],
                            start=(i == 0),
                            stop=(i == ND - 1),
                        )
                    nc.scalar.copy(
                        xtt[g][:, 512 * bj:512 * (bj + 1)], ps[:])

            # ---- recurrence: 4 DVE ops per step, all SBUF -------------
            for t in range(SEQ):
                cols = slice(BSR * t, BSR * (t + 1))
                if t == 0:
                    we = h0_sb[:, 0:BSR]
                    wo = h0_sb[:, BSR:2 * BSR]
                else:
                    we = ybuf[0][:, BSR * (t - 1):BSR * t]
                    wo = ybuf[1][:, BSR * (t - 1):BSR * t]
                nc.vector._custom_dve(
                    ROT, out=zt[0][:], in0=we, in1=wo, s0=A[0], s1=A[1])
                nc.vector._custom_dve(
                    ROT, out=zt[1][:], in0=we, in1=wo, s0=A[2], s1=A[3])
                for g in range(2):
                    nc.vector._custom_dve(
                        MODRELU, out=ybuf[g][:, cols],
                        in0=zt[g][:], in1=xtt[g][:, cols],
                        s0=mv[g], s1=cv[g])
                # stream finished 512-col blocks out to HBM
                if (t + 1) % 32 == 0:
                    blk = (t + 1) // 32 - 1
                    lo, hi = 512 * blk, 512 * (blk + 1)
                    for g in range(2):
                        nc.sync.dma_start(
                            out=y_d[g, :, lo:hi],
                            in_=ybuf[g][:, lo:hi],
                        )

    return nc


def _pack_inputs_rot(x, T, B, bias, h0):
    p = np.arange(NPAIR)
    A00 = B[2 * p, 2 * p]
    A10 = B[2 * p + 1, 2 * p]
    A01 = B[2 * p, 2 * p + 1]
    A11 = B[2 * p + 1, 2 * p + 1]

    maps = []
    for c in range(NCORES):
        cb, cp = divmod(c, NBS)
        q = np.arange(128)
        pe = 128 * cp + q                                 # pair indices
        ue, uo = 2 * pe, 2 * pe + 1                       # unit indices

        Tp = np.ascontiguousarray(
            np.concatenate([T[:, ue], T[:, uo]], axis=1))  # [DIN, 256]

        scal = np.zeros((128, 8), dtype=np.float32)
        scal[:, 0] = A00[pe]
        scal[:, 1] = A10[pe]
        scal[:, 2] = A01[pe]
        scal[:, 3] = A11[pe]
        for g, uu in enumerate((ue, uo)):
            scal[:, 4 + g] = np.where(bias[uu] >= 0, np.float32(1e20),
                                      np.float32(-1.0))
            scal[:, 6 + g] = np.abs(bias[uu])

        h0t = np.concatenate(
            [np.repeat(h0[uu][:, None], BSR, axis=1) for uu in (ue, uo)],
            axis=1).astype(np.float32)                    # [128, 32]

        xs = x[cb * BSR:(cb + 1) * BSR]                   # [BSR, SEQ, DIN]
        xtr = np.ascontiguousarray(
            xs.transpose(2, 1, 0).reshape(DIN, TBR))      # [DIN, (t, b)]
        maps.append({"tmat": Tp, "scal": scal,
                     "h0t": np.ascontiguousarray(h0t), "xtr": xtr})
    return maps


def _unpack_rot(res):
    out = np.empty((SEQ, BATCH, UNITS), dtype=np.float32)
    q = np.arange(128)
    for c in range(NCORES):
        cb, cp = divmod(c, NBS)
        pe = 128 * cp + q
        y = res.results[c]["y"].reshape(2, 128, SEQ, BSR)
        # out[t, 16cb+b, unit(g, q)] = y[g, q, t, b]
        tmp = y.transpose(2, 3, 0, 1)                     # [SEQ, BSR, 2, 128]
        bsl = slice(cb * BSR, (cb + 1) * BSR)
        out[:, bsl, 2 * pe] = tmp[:, :, 0, :]
        out[:, bsl, 2 * pe + 1] = tmp[:, :, 1, :]
    return out


def _is_block_diag(B):
    Babs = np.abs(B)
    mask = np.zeros((UNITS, UNITS), dtype=bool)
    p = np.arange(NPAIR)
    for (i, j) in ((0, 0), (0, 1), (1, 0), (1, 1)):
        mask[2 * p + i, 2 * p + j] = True
    off = Babs[~mask]
    return off.max() <= 1e-5 * max(Babs.max(), 1e-30)


# ---------------------------------------------------------------------------
# Fallback: dense f32r split-precision matmul recurrence (previous kernel)
# ---------------------------------------------------------------------------

def _build_nc_dense(repeat=1):
    nc = bacc.Bacc()

    xtr_d = nc.dram_tensor("xtr", [DIN, TB], F32, kind="ExternalInput")
    t_d = nc.dram_tensor("tmat", [DIN, UNITS], F32, kind="ExternalInput")
    b2_d = nc.dram_tensor("b2", [2 * UNITS, UNITS], F32, kind="ExternalInput")
    h02_d = nc.dram_tensor("h02", [UNITS, 2 * BS], F32, kind="ExternalInput")
    mv_d = nc.dram_tensor("mv", [UNITS, 1], F32, kind="ExternalInput")
    cv_d = nc.dram_tensor("cv", [UNITS, 1], F32, kind="ExternalInput")
    y_d = nc.dram_tensor("y", [NK, 128, TB], F32, kind="ExternalOutput")

    with TileContext(nc) as tc:
        with (
            tc.tile_pool(name="persist", bufs=1) as pp,
            tc.tile_pool(name="pre_psum", bufs=4, space="PSUM") as pre_ps,
            tc.tile_pool(name="rec_psum", bufs=1, space="PSUM") as rec_ps,
        ):
            xtr_sb = [pp.tile([128, TB], F32, tag=f"xtr{i}", name=f"xtr{i}")
                      for i in range(ND)]
            t_sb = [pp.tile([128, UNITS], F32, tag=f"t{i}", name=f"t{i}")
                    for i in range(ND)]
            b2_sb = [pp.tile([128, UNITS], F32R, tag=f"b2_{j}", name=f"b2_{j}")
                     for j in range(2 * NK)]
            h02_sb = pp.tile([128, 2 * BS * NK], F32R, tag="h02", name="h02")
            mv_sb = pp.tile([128, NK], F32, tag="mv", name="mv_sb")
            cv_sb = pp.tile([128, NK], F32, tag="cv", name="cv_sb")
            xtt_all = pp.tile([128, NK * TB], F32, tag="xtt", name="xtt_all")
            ybuf_all = pp.tile([128, NK * TB], F32, tag="ybuf", name="ybuf_all")
            sb2_all = pp.tile([128, 4 * 32], F32R, tag="sb2", name="sb2_all")
            zt_pp = [pp.tile([128, 128], F32, tag=f"zt{i}", name=f"zt{i}")
                     for i in range(2)]
            zs_pp = [pp.tile([128, 32], F32, tag=f"zs{i}", name=f"zs{i}")
                     for i in range(2)]
            ps_pp = [[rec_ps.tile([32, UNITS // 2], F32, tag=f"ps{i}_{h}",
                                  name=f"ps{i}_{h}") for h in range(2)]
                     for i in range(2)]

            xtr_v = [xtr_sb[i][:] for i in range(ND)]
            t_v = [t_sb[i][:] for i in range(ND)]
            bhi_v = [b2_sb[k][:] for k in range(NK)]
            blo_v = [b2_sb[NK + k][:] for k in range(NK)]
            h0_v = [h02_sb[:, 2 * k * BS:2 * (k + 1) * BS] for k in range(NK)]
            mv_v = [mv_sb[:, k:k + 1] for k in range(NK)]
            cv_v = [cv_sb[:, k:k + 1] for k in range(NK)]

            for i in range(ND):
                nc.sync.dma_start(out=xtr_sb[i][:], in_=xtr_d[128 * i:128 * (i + 1), :])
                nc.sync.dma_start(out=t_sb[i][:], in_=t_d[128 * i:128 * (i + 1), :])
            for j in range(2 * NK):
                nc.sync.dma_start(out=b2_sb[j][:],
                                  in_=b2_d[128 * j:128 * (j + 1), :].bitcast(F32R))
            for k in range(NK):
                nc.sync.dma_start(
                    out=h02_sb[:, 2 * k * BS:2 * (k + 1) * BS],
                    in_=h02_d[128 * k:128 * (k + 1), :].bitcast(F32R))
                nc.sync.dma_start(out=mv_sb[:, k:k + 1], in_=mv_d[128 * k:128 * (k + 1), :])
                nc.sync.dma_start(out=cv_sb[:, k:k + 1], in_=cv_d[128 * k:128 * (k + 1), :])

            for i in range(2):
                for h in range(2):
                    nc.vector.memset(ps_pp[i][h][:], 0.0)
            warm_sb = pp.tile([128, 2], F32, tag="warm_sb", name="warm_sb")
            nc.vector.tensor_copy(warm_sb[:, 0:1], mv_v[0])
            nc.vector.tensor_copy(warm_sb[:, 1:2], cv_v[0])

            NJ = TB // 512
            for m in range(NK):
                for j in range(NJ):
                    ps = pre_ps.tile([128, 512], F32, tag="pre", name="pre_ps_t")
                    for i in range(ND):
                        nc.tensor.matmul(
                            ps[:],
                            t_v[i][:, 128 * m:128 * (m + 1)],
                            xtr_v[i][:, 512 * j:512 * (j + 1)],
                            start=(i == 0),
                            stop=(i == ND - 1),
                        )
                    nc.vector.tensor_copy(
                        xtt_all[:, TB * m + 512 * j:TB * m + 512 * (j + 1)], ps[:])

            import contextlib
            loop_cm = (tc.For_i(0, repeat, 1) if repeat > 1
                       else contextlib.nullcontext())
            with loop_cm:
              for t in range(SEQ):
                  zt = zt_pp[t % 2]
                  zs = zs_pp[t % 2]
                  pprev = (t - 1) % 2
                  p = t % 2
                  zt_v = zt[:].rearrange("p (k c) -> p k c", k=NK)
                  zs_v = zs[:].rearrange("p (k c) -> p k c", k=NK)
                  yb_v = ybuf_all[:].rearrange("p (k c) -> p k c", k=NK)
                  sb_v = sb2_all[:].rearrange("p (k c) -> p k c", k=NK)

                  def split_state(ks):
                      yv = yb_v[:, ks, BS * t:BS * (t + 1)]
                      hv = sb_v[:, ks, 16 * p:16 * p + BS]
                      lv = sb_v[:, ks, 16 * p + BS:16 * p + 2 * BS]
                      nc.vector.tensor_copy(hv, yv)
                      nc.vector.tensor_sub(lv, yv, hv.bitcast(F32))

                  for h in range(2):
                      ps = ps_pp[t % 2][h]
                      cols = slice(256 * h, 256 * (h + 1))
                      for k in range(NK):
                          lhsT = (h0_v[k] if t == 0
                                  else sb2_all[:, 32 * k + 16 * pprev:
                                               32 * k + 16 * pprev + 16])
                          for i, bp_v in enumerate((bhi_v, blo_v)):
                              nc.tensor.matmul(
                                  ps[0:2 * BS, :],
                                  lhsT,
                                  bp_v[k][:, cols],
                                  start=(k == 0 and i == 0),
                                  stop=(k == NK - 1 and i == 1),
                              )
                      ps_v = ps[:].rearrange("p (k c) -> p k c", k=2)
                      for q in range(4):
                          in_ap = ps_v[:, :, 32 * q:32 * (q + 1)]
                          out_ap = zt[32 * q:32 * (q + 1), :].rearrange(
                              "p (k c) -> p k c", k=NK)[:, 2 * h:2 * h + 2, :]
                          nc.vector.transpose(out_ap, in_ap)
                      hs = slice(2 * h, 2 * h + 2)
                      nc.vector.tensor_add(
                          zs_v[:, hs, :], zt_v[:, hs, 0:BS],
                          zt_v[:, hs, BS:2 * BS])
                      for k in (2 * h, 2 * h + 1):
                          nc.vector._custom_dve(
                              MODRELU,
                              out=ybuf_all[:, TB * k + BS * t:TB * k + BS * (t + 1)],
                              in0=zs[:, BS * k:BS * (k + 1)],
                              in1=xtt_all[:, TB * k + BS * t:TB * k + BS * (t + 1)],
                              s0=mv_v[k],
                              s1=cv_v[k],
                          )
                      split_state(hs)
                  if (t + 1) % 128 == 0:
                      blk = (t + 1) // 128 - 1
                      lo, hi = 1024 * blk, 1024 * (blk + 1)
                      for k in range(NK):
                          nc.sync.dma_start(
                              out=y_d[k, :, lo:hi],
                              in_=ybuf_all[:, TB * k + lo:TB * k + hi],
                          )

    return nc


def _round_f32r(a):
    u = np.ascontiguousarray(a, dtype=np.float32).view(np.uint32).copy()
    u += np.uint32(0x7FF) + ((u >> np.uint32(12)) & np.uint32(1))
    u &= np.uint32(0xFFFFF000)
    return u.view(np.float32)


def _pack_inputs_dense(x, T, B, bias, h0):
    Bhi = _round_f32r(B)
    Blo = _round_f32r(B - Bhi)
    b2 = np.concatenate([Bhi, Blo], axis=0)
    mv = np.where(bias >= 0, np.float32(1e20), np.float32(-1.0)).astype(np.float32)
    cv = np.abs(bias).astype(np.float32)
    h0b = np.repeat(h0[:, None], BS, axis=1).astype(np.float32)
    h0hi = _round_f32r(h0b)
    h0lo = _round_f32r(h0b - h0hi)
    h02 = np.concatenate([h0hi, h0lo], axis=1)

    base = {
        "tmat": T,
        "b2": b2,
        "h02": np.ascontiguousarray(h02),
        "mv": mv.reshape(UNITS, 1),
        "cv": cv.reshape(UNITS, 1),
    }
    maps = []
    for c in range(NCORES):
        xs = x[c * BS:(c + 1) * BS]
        xtr = np.ascontiguousarray(
            xs.transpose(2, 1, 0).reshape(DIN, TB))
        m = dict(base)
        m["xtr"] = xtr
        maps.append(m)
    return maps


def _unpack_dense(res):
    out = np.empty((SEQ, BATCH, UNITS), dtype=np.float32)
    for c in range(NCORES):
        y = res.results[c]["y"].reshape(NK, 128, SEQ, BS)
        out[:, c * BS:(c + 1) * BS, :] = (
            y.transpose(2, 3, 0, 1).reshape(SEQ, BS, UNITS))
    return out


# ---------------------------------------------------------------------------

def _get_nc(kind):
    if kind not in _NC_CACHE:
        nc = _build_nc_rot() if kind == "rot" else _build_nc_dense()
        nc.finalize()
        _NC_CACHE[kind] = nc
    return _NC_CACHE[kind]


def kernel(x, T, B, bias, h0):
    """Full-input, full-output entry point."""
    global LAST_RESULTS
    x = np.ascontiguousarray(np.asarray(x, dtype=np.float32))
    T = np.ascontiguousarray(np.asarray(T, dtype=np.float32))
    B = np.ascontiguousarray(np.asarray(B, dtype=np.float32))
    bias = np.asarray(bias, dtype=np.float32)
    h0 = np.asarray(h0, dtype=np.float32)

    kind = "rot" if _is_block_diag(B) else "dense"
    if kind == "rot":
        in_maps = _pack_inputs_rot(x, T, B, bias, h0)
    else:
        in_maps = _pack_inputs_dense(x, T, B, bias, h0)

    nc = _get_nc(kind)
    trace = bool(int(os.environ.get("KERNEL_TRACE", "0")))
    res = bass_utils.run_bass_kernel_spmd(
        nc, in_maps, list(range(NCORES)), trace=trace)
    LAST_RESULTS = res

    return _unpack_rot(res) if kind == "rot" else _unpack_dense(res)


if __name__ == "__main__":
    rng = np.random.default_rng(0)
    x = rng.standard_normal((BATCH, SEQ, DIN), dtype=np.float32)
    T = rng.standard_normal((DIN, UNITS), dtype=np.float32) / DIN
    # block-diagonal rotation B
    th = rng.uniform(0, np.pi / 2, NPAIR).astype(np.float32)
    B = np.zeros((UNITS, UNITS), dtype=np.float32)
    p = np.arange(NPAIR)
    B[2 * p, 2 * p] = np.cos(th)
    B[2 * p, 2 * p + 1] = np.sin(th)
    B[2 * p + 1, 2 * p] = -np.sin(th)
    B[2 * p + 1, 2 * p + 1] = np.cos(th)
    bias = rng.uniform(-0.01, 0.01, UNITS).astype(np.float32)
    h0 = np.zeros(UNITS, dtype=np.float32)
    out = kernel(x=x, T=T, B=B, bias=bias, h0=h0)
    print("out", out.shape, out.dtype, float(np.abs(out).mean()))
